# revision 44
# baseline (speedup 1.0000x reference)
"""AttnBlock (GroupNorm + single-head self-attention + residual) on 8 TRN2 cores.

Strategy: data-parallel over batch (b=8) — one NeuronCore per batch element,
no collectives. Per core, everything is computed in [c, pos] layout so no
transposes are needed anywhere.

v2/v3 (current, USE_V2): all big matmuls in fp8e4 DoubleRow mode (2
K-subtiles per instruction, ~1.6x bf16 MAC throughput measured on HW):

  - GroupNorm stats (fp32): free-dim DVE reductions + tiny grouping matmuls.
  - h/q/k/v^T/E/att stored fp8 in SBUF (no DRAM spills). Weights
    host-prescaled by SW=16 into fp8's dynamic range, unscaled for free on
    the PSUM evacuations (activation/tensor_scalar scale).
  - scores computed transposed (S_T[k_pos, q_pos] = k^T q); exp on the
    PSUM->SBUF evacuation computes exp(s/sqrt(c) - 3): the offset keeps E
    inside fp8e4 range (true max logit ~7.2 < ln(240)+3) and cancels between
    numerator and denominator.
  - softmax denominators via a DoubleRow ones-matmul over the E tiles
    (value 1/SA so the reciprocal yields SA/sum(E); att = SA*attnout keeps
    the fp8 att cast in range; outproj evacuation divides by SW*SA).
  - attn@v lands directly in [c, q_pos] layout: lhsT=v^T tiles, rhs=E tiles.
  - phase 3 is software-pipelined (phase3_v3): superblock i's ScalarE-bound
    scores+exp stream interleaves in program order with superblock i-1's
    PE-bound sums/attnv/outproj stream; the v^T projection is deferred from
    phase 2 and interleaved with superblock 0's scores.

v1 (fallback): float32r/bf16 pipeline with q spilled to DRAM.
"""
import sys

sys.path.insert(0, '/opt/trn_rl_repo')

import numpy as np
import ml_dtypes

import concourse.mybir as mybir
import concourse.tile as tile
from concourse import bacc, bass_utils

P = 128
F32 = mybir.dt.float32
F32R = mybir.dt.float32r
BF16 = mybir.dt.bfloat16
FP8 = mybir.dt.float8e4
DR = mybir.MatmulPerfMode.DoubleRow
AF = mybir.ActivationFunctionType
ALU = mybir.AluOpType

# dtype of the big matmuls (weights, h, q/k/v, E):
MM_DT = BF16
MM_NP = ml_dtypes.bfloat16

USE_V2 = True          # fp8 DoubleRow kernel (v2) vs bf16 kernel (v1)
USE_V3_P3 = True       # software-pipelined phase 3 (interleave supers)
V_IN_P3 = False        # defer v^T projection into phase 3 (else in phase 2;
                       # measured slower on HW: 440us vs 403us)
SPLIT_EXP = False      # scores psum as 2x[P,qh] tiles, exp per bank
GN_ON_GPSIMD = True    # GroupNorm apply on the (otherwise idle) Pool engine
SUMS_IN_SCORES = False  # stream denominator sums 2-kt behind the exps inside
                        # the scores loop — measured 420us (fast window) vs
                        # 413.8us for GN_ON_GPSIMD-only (slowest window):
                        # mode-adjusted regression, keep off
SW = 16.0              # host-side weight prescale (fp8 dynamic range)
SA = 32.0              # att prescale (folded into the ones matrix + outproj)
EXP_BIAS = -3.0        # exp(s + EXP_BIAS): keeps E within fp8e4 range


def build_attn_kernel(c=512, n=4096, groups=32, eps=1e-6, pb=512, qb=256,
                      mm_dt=MM_DT, stop_after=None, repeat=1):
    if USE_V2:
        return build_attn_kernel_v2(c=c, n=n, groups=groups, eps=eps, pb=pb,
                                    stop_after=stop_after, repeat=repeat)
    return build_attn_kernel_v1(c=c, n=n, groups=groups, eps=eps, pb=pb,
                                qb=qb, mm_dt=mm_dt, stop_after=stop_after,
                                repeat=repeat)


def build_attn_kernel_v2(c=512, n=4096, groups=32, eps=1e-6, pb=512,
                         qsb=1024, stop_after=None, repeat=1):
    """fp8 DoubleRow kernel: q/k/v/E/att in fp8e4, all big matmuls in
    DoubleRow mode (2 K-subtiles per instruction). Weights host-prescaled by
    SW; att prescaled by SA (folded into the 1/SA ones matrix); exp computed
    as exp(s/sqrt(c) + EXP_BIAS) which cancels between numerator and
    denominator. No DRAM q spill: q/k/v^T/E live in SBUF."""
    cs = c // P                 # 4 channel subtiles
    nbp = n // pb               # projection-phase position blocks
    nsb = n // qsb              # attention q superblocks
    qh = qsb // 2               # half superblock = one PSUM bank
    kt_n = n // P               # k-position tiles (32)
    gps = P // (c // groups)    # groups per channel-subtile (8)

    nc = bacc.Bacc("TRN2", target_bir_lowering=False, debug=False,
                   enable_asserts=False)

    x_d = nc.dram_tensor("x", (c, n), F32, kind="ExternalInput").ap()
    wq_d = nc.dram_tensor("wqt", (c, c), FP8, kind="ExternalInput").ap()
    wk_d = nc.dram_tensor("wkt", (c, c), FP8, kind="ExternalInput").ap()
    wv_d = nc.dram_tensor("wvt", (c, c), FP8, kind="ExternalInput").ap()
    wp_d = nc.dram_tensor("wpt", (c, c), FP8, kind="ExternalInput").ap()
    bqs_d = nc.dram_tensor("bqs", (P, cs), F32, kind="ExternalInput").ap()
    bks_d = nc.dram_tensor("bks", (P, cs), F32, kind="ExternalInput").ap()
    bps_d = nc.dram_tensor("bps", (P, cs), F32, kind="ExternalInput").ap()
    gws_d = nc.dram_tensor("gws", (P, cs), F32, kind="ExternalInput").ap()
    gbs_d = nc.dram_tensor("gbs", (P, cs), F32, kind="ExternalInput").ap()
    gm_d = nc.dram_tensor("gmat", (P, gps), F32, kind="ExternalInput").ap()
    gm2_d = nc.dram_tensor("gmat2", (gps, P), F32, kind="ExternalInput").ap()
    out_d = nc.dram_tensor("out", (c, n), F32, kind="ExternalOutput").ap()

    with tile.TileContext(nc) as tc:
        cpool = tc.alloc_tile_pool(name="const", bufs=1)
        wpool = tc.alloc_tile_pool(name="w8", bufs=1)
        qpool = tc.alloc_tile_pool(name="qres", bufs=1)
        kpool = tc.alloc_tile_pool(name="kres", bufs=1)
        vpool = tc.alloc_tile_pool(name="vres", bufs=1)

        wq8 = wpool.tile([P, cs, c], FP8)
        wk8 = wpool.tile([P, cs, c], FP8)
        wv8 = wpool.tile([P, cs, c], FP8)
        wp8 = wpool.tile([P, cs, c], FP8)
        for t, d in ((wq8, wq_d), (wk8, wk_d), (wv8, wv_d), (wp8, wp_d)):
            nc.sync.dma_start(t[:], d.rearrange("(ci p) o -> p ci o", p=P))

        q_sb = qpool.tile([P, cs, n], FP8)
        k_sb = kpool.tile([P, cs, n], FP8)
        vt_sb = vpool.tile([P, kt_n, c], FP8)
        h_sb = vpool.tile([P, cs, n], FP8)

        bqs = cpool.tile([P, cs], F32)
        bks = cpool.tile([P, cs], F32)
        bps = cpool.tile([P, cs], F32)
        gws = cpool.tile([P, cs], F32)
        gbs = cpool.tile([P, cs], F32)
        gm = cpool.tile([P, gps], F32)
        gm2 = cpool.tile([gps, P], F32)
        for t, d in ((bqs, bqs_d), (bks, bks_d), (bps, bps_d), (gws, gws_d),
                     (gbs, gbs_d), (gm, gm_d), (gm2, gm2_d)):
            nc.sync.dma_start(t[:], d)

        ones8 = cpool.tile([P, 2, P], FP8)
        nc.vector.memset(ones8[:], 1.0 / SA)
        expb = cpool.tile([P, 1], F32)
        nc.vector.memset(expb[:], EXP_BIAS)

        # ---------------- Phase 1: GroupNorm statistics ----------------
        pb1 = min(1024, n)
        nb1 = n // pb1
        s1 = cpool.tile([P, cs, nb1], F32)
        s2 = cpool.tile([P, cs, nb1], F32)
        with tc.tile_pool(name="p1", bufs=6) as p1, \
             tc.tile_pool(name="ps1", bufs=2, space="PSUM") as ps1:
            for ci in range(cs):
                for j in range(nb1):
                    xt = p1.tile([P, pb1], F32, tag="x1")
                    nc.sync.dma_start(
                        xt[:], x_d[ci * P:(ci + 1) * P, j * pb1:(j + 1) * pb1])
                    nc.vector.reduce_sum(
                        out=s1[:, ci, j:j + 1], in_=xt[:],
                        axis=mybir.AxisListType.X)
                    sq = p1.tile([P, pb1], F32, tag="sq")
                    nc.scalar.activation(sq[:], xt[:], AF.Square,
                                         accum_out=s2[:, ci, j:j + 1])
            st = cpool.tile([P, 2 * cs], F32)
            nc.vector.reduce_sum(out=st[:, 0:cs], in_=s1[:],
                                 axis=mybir.AxisListType.X)
            nc.vector.reduce_sum(out=st[:, cs:2 * cs], in_=s2[:],
                                 axis=mybir.AxisListType.X)
            psg = ps1.tile([gps, 2 * cs], F32)
            nc.tensor.matmul(psg[:], gm[:], st[:], start=True, stop=True)
            gsb = cpool.tile([gps, 2 * cs], F32)
            nc.vector.tensor_copy(gsb[:], psg[:])
            inv_cnt = 1.0 / (n * (c // groups))
            mean = cpool.tile([gps, cs], F32)
            e2 = cpool.tile([gps, cs], F32)
            nc.vector.tensor_scalar_mul(mean[:], gsb[:, 0:cs], inv_cnt)
            nc.vector.tensor_scalar_mul(e2[:], gsb[:, cs:2 * cs], inv_cnt)
            var = cpool.tile([gps, cs], F32)
            nc.vector.tensor_tensor(var[:], mean[:], mean[:], ALU.mult)
            nc.vector.tensor_tensor(var[:], e2[:], var[:], ALU.subtract)
            std = cpool.tile([gps, cs], F32)
            eps_t = cpool.tile([P, 1], F32)
            nc.vector.memset(eps_t[:], float(eps))
            nc.scalar.activation(std[:], var[:], AF.Sqrt, bias=eps_t[:gps, :])
            st2 = cpool.tile([gps, 2 * cs], F32)
            nc.vector.reciprocal(st2[:, 0:cs], std[:])
            nc.vector.tensor_tensor(st2[:, cs:2 * cs], mean[:], st2[:, 0:cs],
                                    ALU.mult)
            nc.vector.tensor_scalar_mul(st2[:, cs:2 * cs], st2[:, cs:2 * cs],
                                        -1.0)
            psb = ps1.tile([P, 2 * cs], F32)
            nc.tensor.matmul(psb[:], gm2[:], st2[:], start=True, stop=True)
            bc = cpool.tile([P, 2 * cs], F32)
            nc.vector.tensor_copy(bc[:], psb[:])
            a_ch = cpool.tile([P, cs], F32)
            b_ch = cpool.tile([P, cs], F32)
            nc.vector.tensor_tensor(a_ch[:], gws[:], bc[:, 0:cs], ALU.mult)
            nc.vector.tensor_tensor(b_ch[:], gws[:], bc[:, cs:2 * cs], ALU.mult)
            nc.vector.tensor_tensor(b_ch[:], b_ch[:], gbs[:], ALU.add)

        if stop_after == 'p1':
            dbg = cpool.tile([P, 2 * cs], F32)
            nc.vector.tensor_copy(dbg[:, 0:cs], a_ch[:])
            nc.vector.tensor_copy(dbg[:, cs:2 * cs], b_ch[:])
            nc.sync.dma_start(out_d[0:P, 0:2 * cs], dbg[:])
        else:
            for _rep in range(repeat):
                if USE_V3_P3:
                    if V_IN_P3:
                        phase2_v3(nc, tc, cs, c, n, pb, x_d, wq8, wk8, q_sb,
                                  k_sb, h_sb, a_ch, b_ch, bqs, bks)
                        if stop_after == 'p2':
                            break
                        phase3_v3(nc, tc, cs, c, n, qsb, x_d, out_d, q_sb,
                                  k_sb, vt_sb, wp8, bps, ones8, expb, h_sb,
                                  wv8)
                    else:
                        phase2_v2(nc, tc, cs, c, n, pb, x_d, wq8, wk8, wv8,
                                  q_sb, k_sb, vt_sb, a_ch, b_ch, bqs, bks)
                        if stop_after == 'p2':
                            break
                        phase3_v3(nc, tc, cs, c, n, qsb, x_d, out_d, q_sb,
                                  k_sb, vt_sb, wp8, bps, ones8, expb, h_sb)
                else:
                    phase2_v2(nc, tc, cs, c, n, pb, x_d, wq8, wk8, wv8, q_sb,
                              k_sb, vt_sb, a_ch, b_ch, bqs, bks)
                    if stop_after == 'p2':
                        break
                    phase3_v2(nc, tc, cs, c, n, qsb, x_d, out_d, q_sb, k_sb,
                              vt_sb, wp8, bps, ones8, expb)
            if stop_after == 'p2':
                with tc.tile_pool(name="dbg2", bufs=2) as dbg2:
                    for ci in range(cs):
                        t = dbg2.tile([P, n], F32, tag="d")
                        nc.vector.tensor_copy(t[:], k_sb[:, ci, :])
                        nc.sync.dma_start(out_d[ci * P:(ci + 1) * P, :], t[:])

        for p in (vpool, kpool, qpool, wpool, cpool):
            p.release()

    nc.finalize()
    return nc


def phase2_v2(nc, tc, cs, c, n, pb, x_d, wq8, wk8, wv8, q_sb, k_sb, vt_sb,
              a_ch, b_ch, bqs, bks):
    nbp = n // pb
    ncp = cs // 2
    with tc.tile_pool(name="p2x", bufs=4) as p2x, \
         tc.tile_pool(name="p2h", bufs=3) as p2h, \
         tc.tile_pool(name="ps2", bufs=4, space="PSUM") as ps2:
        for j in range(nbp):
            js = slice(j * pb, (j + 1) * pb)
            ht = p2h.tile([P, cs, pb], FP8, tag="h")
            for ci in range(cs):
                xt = p2x.tile([P, pb], F32, tag="x2")
                nc.sync.dma_start(xt[:], x_d[ci * P:(ci + 1) * P, js])
                if GN_ON_GPSIMD:
                    nc.gpsimd.tensor_scalar(ht[:, ci, :], xt[:],
                                            a_ch[:, ci:ci + 1],
                                            b_ch[:, ci:ci + 1],
                                            ALU.mult, ALU.add)
                else:
                    nc.scalar.activation(ht[:, ci, :], xt[:], AF.Identity,
                                         bias=b_ch[:, ci:ci + 1],
                                         scale=a_ch[:, ci:ci + 1])
            # q and k evacuations both on ScalarE: measured faster than
            # offloading k to DVE (445us vs 403-420us) — DVE's slower
            # PSUM-evac rate stalls the ps2 rotation and the k->scores chain
            for w8, bias_t, tgt in ((wq8, bqs, q_sb), (wk8, bks, k_sb)):
                for co in range(cs):
                    ps = ps2.tile([P, pb], F32, tag="proj")
                    for cp in range(ncp):
                        nc.tensor.matmul(
                            ps[:], w8[:, 2 * cp:2 * cp + 2, co * P:(co + 1) * P],
                            ht[:, 2 * cp:2 * cp + 2, :],
                            start=(cp == 0), stop=(cp == ncp - 1),
                            perf_mode=DR)
                    nc.scalar.activation(tgt[:, co, js], ps[:], AF.Identity,
                                         bias=bias_t[:, co:co + 1],
                                         scale=1.0 / SW)
            for pt in range(pb // P):
                ps = ps2.tile([P, c], F32, tag="proj")
                for cp in range(ncp):
                    nc.tensor.matmul(
                        ps[:], ht[:, 2 * cp:2 * cp + 2, pt * P:(pt + 1) * P],
                        wv8[:, 2 * cp:2 * cp + 2, :],
                        start=(cp == 0), stop=(cp == ncp - 1), perf_mode=DR)
                nc.vector.tensor_scalar_mul(
                    vt_sb[:, j * (pb // P) + pt, :], ps[:], 1.0 / SW)


def phase3_v2(nc, tc, cs, c, n, qsb, x_d, out_d, q_sb, k_sb, vt_sb, wp8,
              bps, ones8, expb):
    nsb = n // qsb
    qh = qsb // 2
    kt_n = n // P
    ncp = cs // 2
    with tc.tile_pool(name="es", bufs=2) as espool, \
         tc.tile_pool(name="attp", bufs=2) as attp, \
         tc.tile_pool(name="p3s", bufs=3) as p3s, \
         tc.tile_pool(name="p3x", bufs=3) as p3x, \
         tc.tile_pool(name="pss", bufs=2, space="PSUM") as pss, \
         tc.tile_pool(name="psv", bufs=1, space="PSUM") as psv, \
         tc.tile_pool(name="psm", bufs=1, space="PSUM") as psm:
        for si in range(nsb):
            g0 = slice(si * qsb, si * qsb + qh)
            g1 = slice(si * qsb + qh, (si + 1) * qsb)
            es_t = espool.tile([P, kt_n, qsb], FP8, tag="es")
            # scores^T in two PSUM banks (q halves) -> exp -> fp8 E
            for kt in range(kt_n):
                ps_s = pss.tile([P, qsb], F32, tag="s")
                for cp in range(ncp):
                    st, sp = (cp == 0), (cp == ncp - 1)
                    w = k_sb[:, 2 * cp:2 * cp + 2, kt * P:(kt + 1) * P]
                    nc.tensor.matmul(ps_s[:, 0:qh], w,
                                     q_sb[:, 2 * cp:2 * cp + 2, g0],
                                     start=st, stop=sp, perf_mode=DR)
                    nc.tensor.matmul(ps_s[:, qh:qsb], w,
                                     q_sb[:, 2 * cp:2 * cp + 2, g1],
                                     start=st, stop=sp, perf_mode=DR)
                nc.scalar.activation(es_t[:, kt, :], ps_s[:], AF.Exp,
                                     bias=expb[:], scale=c ** -0.5)
            # softmax denominators: (1/SA) * sum_k E  (ones matmul over kt)
            ps_d = psm.tile([P, qsb], F32, tag="d")
            for ktp in range(kt_n // 2):
                st, sp = (ktp == 0), (ktp == kt_n // 2 - 1)
                nc.tensor.matmul(ps_d[:, 0:qh], ones8[:],
                                 es_t[:, 2 * ktp:2 * ktp + 2, 0:qh],
                                 start=st, stop=sp, perf_mode=DR)
                nc.tensor.matmul(ps_d[:, qh:qsb], ones8[:],
                                 es_t[:, 2 * ktp:2 * ktp + 2, qh:qsb],
                                 start=st, stop=sp, perf_mode=DR)
            rrep = p3s.tile([P, qsb], F32, tag="rrep")
            nc.vector.reciprocal(rrep[:], ps_d[:])      # = SA / sum(E)
            # attn @ v -> att = SA * attnout (fp8)
            att = attp.tile([P, cs, qsb], FP8, tag="att")
            for co in range(cs):
                ps_o = psv.tile([P, qsb], F32, tag="o")
                for ktp in range(kt_n // 2):
                    st, sp = (ktp == 0), (ktp == kt_n // 2 - 1)
                    w = vt_sb[:, 2 * ktp:2 * ktp + 2, co * P:(co + 1) * P]
                    nc.tensor.matmul(ps_o[:, 0:qh], w,
                                     es_t[:, 2 * ktp:2 * ktp + 2, 0:qh],
                                     start=st, stop=sp, perf_mode=DR)
                    nc.tensor.matmul(ps_o[:, qh:qsb], w,
                                     es_t[:, 2 * ktp:2 * ktp + 2, qh:qsb],
                                     start=st, stop=sp, perf_mode=DR)
                nc.vector.tensor_tensor(att[:, co, :], ps_o[:], rrep[:],
                                        ALU.mult)
            # output projection + bias + residual (psum buffer shared with
            # the denominator sums: d(i) -> p(i) -> d(i+1) never overlap)
            for co in range(cs):
                ps_p = psm.tile([P, qsb], F32, tag="d")
                for cp in range(ncp):
                    st, sp = (cp == 0), (cp == ncp - 1)
                    w = wp8[:, 2 * cp:2 * cp + 2, co * P:(co + 1) * P]
                    nc.tensor.matmul(ps_p[:, 0:qh], w,
                                     att[:, 2 * cp:2 * cp + 2, 0:qh],
                                     start=st, stop=sp, perf_mode=DR)
                    nc.tensor.matmul(ps_p[:, qh:qsb], w,
                                     att[:, 2 * cp:2 * cp + 2, qh:qsb],
                                     start=st, stop=sp, perf_mode=DR)
                xr = p3x.tile([P, qsb], F32, tag="xr")
                nc.sync.dma_start(
                    xr[:], x_d[co * P:(co + 1) * P, si * qsb:(si + 1) * qsb])
                t1 = p3s.tile([P, qsb], F32, tag="t1")
                nc.scalar.activation(t1[:], ps_p[:], AF.Identity,
                                     bias=bps[:, co:co + 1],
                                     scale=1.0 / (SW * SA))
                ot = p3s.tile([P, qsb], F32, tag="ot")
                nc.vector.tensor_tensor(ot[:], t1[:], xr[:], ALU.add)
                nc.sync.dma_start(
                    out_d[co * P:(co + 1) * P, si * qsb:(si + 1) * qsb], ot[:])


def phase2_v3(nc, tc, cs, c, n, pb, x_d, wq8, wk8, q_sb, k_sb, h_sb, a_ch,
              b_ch, bqs, bks, wv8=None, vt_sb=None):
    """GN apply + q/k projections. h is SBUF-resident; the v^T projection is
    deferred into phase 3 where it interleaves with superblock 0's scores
    (hiding its PE work under the otherwise ScalarE-bound exp stream). k's
    evacuation runs on DVE to unload ScalarE."""
    nbp = n // pb
    ncp = cs // 2
    with tc.tile_pool(name="p2x", bufs=4) as p2x, \
         tc.tile_pool(name="ps2", bufs=4, space="PSUM") as ps2:
        for j in range(nbp):
            js = slice(j * pb, (j + 1) * pb)
            for ci in range(cs):
                xt = p2x.tile([P, pb], F32, tag="x2")
                nc.sync.dma_start(xt[:], x_d[ci * P:(ci + 1) * P, js])
                nc.scalar.activation(h_sb[:, ci, js], xt[:], AF.Identity,
                                     bias=b_ch[:, ci:ci + 1],
                                     scale=a_ch[:, ci:ci + 1])
            for co in range(cs):
                ps = ps2.tile([P, pb], F32, tag="proj")
                for cp in range(ncp):
                    nc.tensor.matmul(
                        ps[:], wq8[:, 2 * cp:2 * cp + 2, co * P:(co + 1) * P],
                        h_sb[:, 2 * cp:2 * cp + 2, js],
                        start=(cp == 0), stop=(cp == ncp - 1), perf_mode=DR)
                nc.scalar.activation(q_sb[:, co, js], ps[:], AF.Identity,
                                     bias=bqs[:, co:co + 1], scale=1.0 / SW)
            for co in range(cs):
                ps = ps2.tile([P, pb], F32, tag="proj")
                for cp in range(ncp):
                    nc.tensor.matmul(
                        ps[:], wk8[:, 2 * cp:2 * cp + 2, co * P:(co + 1) * P],
                        h_sb[:, 2 * cp:2 * cp + 2, js],
                        start=(cp == 0), stop=(cp == ncp - 1), perf_mode=DR)
                nc.vector.tensor_scalar(k_sb[:, co, js], ps[:], 1.0 / SW,
                                        bks[:, co:co + 1], ALU.mult, ALU.add)
            if wv8 is not None:
                for pt in range(pb // P):
                    ps = ps2.tile([P, c], F32, tag="proj")
                    base = j * pb + pt * P
                    for cp in range(ncp):
                        nc.tensor.matmul(
                            ps[:], h_sb[:, 2 * cp:2 * cp + 2, base:base + P],
                            wv8[:, 2 * cp:2 * cp + 2, :],
                            start=(cp == 0), stop=(cp == ncp - 1),
                            perf_mode=DR)
                    nc.vector.tensor_scalar_mul(
                        vt_sb[:, j * (pb // P) + pt, :], ps[:], 1.0 / SW)


def phase3_v3(nc, tc, cs, c, n, qsb, x_d, out_d, q_sb, k_sb, vt_sb, wp8,
              bps, ones8, expb, h_sb, wv8=None):
    """Software-pipelined phase 3: the scores+exp stream of superblock i
    (ScalarE-heavy) is interleaved in program order with the sums/attnv/
    outproj stream of superblock i-1 (PE-heavy), so the exp evacuations hide
    under attention matmuls instead of serializing with the scores. The v^T
    projection (moved out of phase 2) interleaves with superblock 0."""
    nsb = n // qsb
    qh = qsb // 2
    kt_n = n // P
    ncp = cs // 2
    with tc.tile_pool(name="es", bufs=2) as espool, \
         tc.tile_pool(name="attp", bufs=2) as attp, \
         tc.tile_pool(name="p3s", bufs=3) as p3s, \
         tc.tile_pool(name="p3x", bufs=3) as p3x, \
         tc.tile_pool(name="pss", bufs=2, space="PSUM") as pss, \
         tc.tile_pool(name="psv", bufs=1, space="PSUM") as psv, \
         tc.tile_pool(name="psm", bufs=1, space="PSUM") as psm:

        def gen_scores(si, es_t, holder):
            g0 = slice(si * qsb, si * qsb + qh)
            g1 = slice(si * qsb + qh, (si + 1) * qsb)
            for kt in range(kt_n):
                if SPLIT_EXP:
                    ps_a = pss.tile([P, qh], F32, tag="s")
                    ps_b = pss.tile([P, qh], F32, tag="sB")
                else:
                    ps_s = pss.tile([P, qsb], F32, tag="s")
                    ps_a, ps_b = ps_s[:, 0:qh], ps_s[:, qh:qsb]
                for cp in range(ncp):
                    st, sp = (cp == 0), (cp == ncp - 1)
                    w = k_sb[:, 2 * cp:2 * cp + 2, kt * P:(kt + 1) * P]
                    nc.tensor.matmul(ps_a[:], w,
                                     q_sb[:, 2 * cp:2 * cp + 2, g0],
                                     start=st, stop=sp, perf_mode=DR)
                    nc.tensor.matmul(ps_b[:], w,
                                     q_sb[:, 2 * cp:2 * cp + 2, g1],
                                     start=st, stop=sp, perf_mode=DR)
                if SPLIT_EXP:
                    nc.scalar.activation(es_t[:, kt, 0:qh], ps_a[:], AF.Exp,
                                         bias=expb[:], scale=c ** -0.5)
                    nc.scalar.activation(es_t[:, kt, qh:qsb], ps_b[:], AF.Exp,
                                         bias=expb[:], scale=c ** -0.5)
                else:
                    nc.scalar.activation(es_t[:, kt, :], ps_s[:], AF.Exp,
                                         bias=expb[:], scale=c ** -0.5)
                if SUMS_IN_SCORES and kt % 2 == 1:
                    ktp = (kt - 1) // 2
                    if ktp == 0:
                        holder[0] = psm.tile([P, qsb], F32, tag="d",
                                             name="ps_d")
                    ps_d = holder[0]
                    st, sp = (ktp == 0), (ktp == kt_n // 2 - 1)
                    nc.tensor.matmul(ps_d[:, 0:qh], ones8[:],
                                     es_t[:, kt - 1:kt + 1, 0:qh],
                                     start=st, stop=sp, perf_mode=DR)
                    nc.tensor.matmul(ps_d[:, qh:qsb], ones8[:],
                                     es_t[:, kt - 1:kt + 1, qh:qsb],
                                     start=st, stop=sp, perf_mode=DR)
                yield

        def gen_v():
            pbv = 512
            idx = 0
            for j in range(n // pbv):
                for pt in range(pbv // P):
                    # alternate between the two (otherwise idle) psum pools
                    # so v chunks double-buffer instead of serializing on
                    # their DVE evacuations
                    pool, tg = (psm, "d") if idx % 2 == 0 else (psv, "o")
                    idx += 1
                    ps = pool.tile([P, c], F32, tag=tg)
                    base = j * pbv + pt * P
                    for cp in range(ncp):
                        nc.tensor.matmul(
                            ps[:], h_sb[:, 2 * cp:2 * cp + 2, base:base + P],
                            wv8[:, 2 * cp:2 * cp + 2, :],
                            start=(cp == 0), stop=(cp == ncp - 1),
                            perf_mode=DR)
                    nc.vector.tensor_scalar_mul(
                        vt_sb[:, j * (pbv // P) + pt, :], ps[:], 1.0 / SW)
                    yield

        def gen_rest(si, es_t, holder):
            if SUMS_IN_SCORES:
                ps_d = holder[0]
            else:
                ps_d = psm.tile([P, qsb], F32, tag="d")
                for ktp in range(kt_n // 2):
                    st, sp = (ktp == 0), (ktp == kt_n // 2 - 1)
                    nc.tensor.matmul(ps_d[:, 0:qh], ones8[:],
                                     es_t[:, 2 * ktp:2 * ktp + 2, 0:qh],
                                     start=st, stop=sp, perf_mode=DR)
                    nc.tensor.matmul(ps_d[:, qh:qsb], ones8[:],
                                     es_t[:, 2 * ktp:2 * ktp + 2, qh:qsb],
                                     start=st, stop=sp, perf_mode=DR)
                    yield
            rrep = p3s.tile([P, qsb], F32, tag="rrep")
            nc.vector.reciprocal(rrep[:], ps_d[:])      # = SA / sum(E)
            yield
            att = attp.tile([P, cs, qsb], FP8, tag="att")
            for co in range(cs):
                ps_o = psv.tile([P, qsb], F32, tag="o")
                for ktp in range(kt_n // 2):
                    st, sp = (ktp == 0), (ktp == kt_n // 2 - 1)
                    w = vt_sb[:, 2 * ktp:2 * ktp + 2, co * P:(co + 1) * P]
                    nc.tensor.matmul(ps_o[:, 0:qh], w,
                                     es_t[:, 2 * ktp:2 * ktp + 2, 0:qh],
                                     start=st, stop=sp, perf_mode=DR)
                    nc.tensor.matmul(ps_o[:, qh:qsb], w,
                                     es_t[:, 2 * ktp:2 * ktp + 2, qh:qsb],
                                     start=st, stop=sp, perf_mode=DR)
                    yield
                nc.vector.tensor_tensor(att[:, co, :], ps_o[:], rrep[:],
                                        ALU.mult)
                yield
            for co in range(cs):
                # with sums streamed into the scores loop, the next super's
                # sums own psm — outproj shares attnv's psum instead (its
                # data deps already serialize it behind the last normalize)
                if SUMS_IN_SCORES:
                    ps_p = psv.tile([P, qsb], F32, tag="o")
                else:
                    ps_p = psm.tile([P, qsb], F32, tag="d")
                for cp in range(ncp):
                    st, sp = (cp == 0), (cp == ncp - 1)
                    w = wp8[:, 2 * cp:2 * cp + 2, co * P:(co + 1) * P]
                    nc.tensor.matmul(ps_p[:, 0:qh], w,
                                     att[:, 2 * cp:2 * cp + 2, 0:qh],
                                     start=st, stop=sp, perf_mode=DR)
                    nc.tensor.matmul(ps_p[:, qh:qsb], w,
                                     att[:, 2 * cp:2 * cp + 2, qh:qsb],
                                     start=st, stop=sp, perf_mode=DR)
                xr = p3x.tile([P, qsb], F32, tag="xr")
                nc.sync.dma_start(
                    xr[:], x_d[co * P:(co + 1) * P, si * qsb:(si + 1) * qsb])
                t1 = p3s.tile([P, qsb], F32, tag="t1")
                nc.scalar.activation(t1[:], ps_p[:], AF.Identity,
                                     bias=bps[:, co:co + 1],
                                     scale=1.0 / (SW * SA))
                ot = p3s.tile([P, qsb], F32, tag="ot")
                nc.vector.tensor_tensor(ot[:], t1[:], xr[:], ALU.add)
                nc.sync.dma_start(
                    out_d[co * P:(co + 1) * P, si * qsb:(si + 1) * qsb],
                    ot[:])
                yield

        es_tiles = []
        rest = None
        vgen = gen_v() if wv8 is not None else None
        for si in range(nsb):
            es_t = espool.tile([P, kt_n, qsb], FP8, tag="es")
            es_tiles.append(es_t)
            holder = [None]
            scores = gen_scores(si, es_t, holder)
            for _ in scores:
                if rest is not None:
                    for _r in range(3):
                        try:
                            next(rest)
                        except StopIteration:
                            rest = None
                            break
                elif vgen is not None:
                    try:
                        next(vgen)
                    except StopIteration:
                        vgen = None
            if vgen is not None:
                for _ in vgen:
                    pass
                vgen = None
            if rest is not None:
                for _ in rest:
                    pass
            rest = gen_rest(si, es_t, holder)
        for _ in rest:
            pass


def build_attn_kernel_v1(c=512, n=4096, groups=32, eps=1e-6, pb=512, qb=256,
                         mm_dt=MM_DT, stop_after=None, repeat=1):
    """Build the per-core Bass program. Returns finalized nc.

    stop_after: None (full), 'p1' (GN stats only), 'p2' (through projections)
    — debug bisection: later phases are skipped and 'out' is filled from
    whatever is available.
    """
    cs = c // P                 # channel subtiles (4)
    nbp = n // pb               # projection-phase position blocks
    nqb = n // qb               # attention q blocks
    kt_n = n // P               # k-position tiles (32)
    gps = P // (c // groups)    # groups per channel-subtile (8)

    nc = bacc.Bacc("TRN2", target_bir_lowering=False, debug=False,
                   enable_asserts=False)

    x_d = nc.dram_tensor("x", (c, n), F32, kind="ExternalInput").ap()
    wq_d = nc.dram_tensor("wqt", (c, c), mm_dt, kind="ExternalInput").ap()
    wk_d = nc.dram_tensor("wkt", (c, c), mm_dt, kind="ExternalInput").ap()
    wv_d = nc.dram_tensor("wvt", (c, c), mm_dt, kind="ExternalInput").ap()
    wp_d = nc.dram_tensor("wpt", (c, c), mm_dt, kind="ExternalInput").ap()
    bqs_d = nc.dram_tensor("bqs", (P, cs), F32, kind="ExternalInput").ap()
    bks_d = nc.dram_tensor("bks", (P, cs), F32, kind="ExternalInput").ap()
    bps_d = nc.dram_tensor("bps", (P, cs), F32, kind="ExternalInput").ap()
    gws_d = nc.dram_tensor("gws", (P, cs), F32, kind="ExternalInput").ap()
    gbs_d = nc.dram_tensor("gbs", (P, cs), F32, kind="ExternalInput").ap()
    gm_d = nc.dram_tensor("gmat", (P, gps), F32, kind="ExternalInput").ap()
    gm2_d = nc.dram_tensor("gmat2", (gps, P), F32, kind="ExternalInput").ap()
    out_d = nc.dram_tensor("out", (c, n), F32, kind="ExternalOutput").ap()

    with tile.TileContext(nc) as tc:
        cpool = tc.alloc_tile_pool(name="const", bufs=1)
        kpool = tc.alloc_tile_pool(name="kfull", bufs=1)
        vpool = tc.alloc_tile_pool(name="vtfull", bufs=1)
        wppool = tc.alloc_tile_pool(name="wppre", bufs=1)
        dpool = tc.alloc_tile_pool(name="dram", bufs=1, space="DRAM")
        wp_t = wppool.tile([P, cs, c], mm_dt)

        q_dram = dpool.tile([P, cs, n], mm_dt)
        k_full = kpool.tile([P, cs, n], mm_dt)
        vt_full = vpool.tile([P, kt_n, c], mm_dt)

        bqs = cpool.tile([P, cs], F32)
        bks = cpool.tile([P, cs], F32)
        bps = cpool.tile([P, cs], F32)
        gws = cpool.tile([P, cs], F32)
        gbs = cpool.tile([P, cs], F32)
        gm = cpool.tile([P, gps], F32)
        gm2 = cpool.tile([gps, P], F32)
        for t, d in ((bqs, bqs_d), (bks, bks_d), (bps, bps_d), (gws, gws_d),
                     (gbs, gbs_d), (gm, gm_d), (gm2, gm2_d)):
            nc.sync.dma_start(t[:], d)

        ones_mat = cpool.tile([P, P], F32)
        nc.vector.memset(ones_mat[:], 1.0)

        # ---------------- Phase 1: GroupNorm statistics ----------------
        pb1 = min(1024, n)      # big DMA tiles: keep all queues busy
        nb1 = n // pb1
        s1 = cpool.tile([P, cs, nb1], F32)
        s2 = cpool.tile([P, cs, nb1], F32)
        with tc.tile_pool(name="p1", bufs=6) as p1, \
             tc.tile_pool(name="ps1", bufs=2, space="PSUM") as ps1:
            for ci in range(cs):
                for j in range(nb1):
                    xt = p1.tile([P, pb1], F32, tag="x1")
                    nc.sync.dma_start(
                        xt[:], x_d[ci * P:(ci + 1) * P, j * pb1:(j + 1) * pb1])
                    nc.vector.reduce_sum(
                        out=s1[:, ci, j:j + 1], in_=xt[:],
                        axis=mybir.AxisListType.X)
                    sq = p1.tile([P, pb1], F32, tag="sq")
                    nc.scalar.activation(sq[:], xt[:], AF.Square,
                                         accum_out=s2[:, ci, j:j + 1])
            # per-(channel, ci) totals
            st = cpool.tile([P, 2 * cs], F32)
            nc.vector.reduce_sum(out=st[:, 0:cs], in_=s1[:],
                                 axis=mybir.AxisListType.X)
            nc.vector.reduce_sum(out=st[:, cs:2 * cs], in_=s2[:],
                                 axis=mybir.AxisListType.X)
            # group sums across partitions: [gps, 2cs] = gm^T @ st
            psg = ps1.tile([gps, 2 * cs], F32)
            nc.tensor.matmul(psg[:], gm[:], st[:], start=True, stop=True)
            gsb = cpool.tile([gps, 2 * cs], F32)
            nc.vector.tensor_copy(gsb[:], psg[:])
            inv_cnt = 1.0 / (n * (c // groups))
            mean = cpool.tile([gps, cs], F32)
            e2 = cpool.tile([gps, cs], F32)
            nc.vector.tensor_scalar_mul(mean[:], gsb[:, 0:cs], inv_cnt)
            nc.vector.tensor_scalar_mul(e2[:], gsb[:, cs:2 * cs], inv_cnt)
            var = cpool.tile([gps, cs], F32)
            nc.vector.tensor_tensor(var[:], mean[:], mean[:], ALU.mult)
            nc.vector.tensor_tensor(var[:], e2[:], var[:], ALU.subtract)
            std = cpool.tile([gps, cs], F32)
            eps_t = cpool.tile([P, 1], F32)
            nc.vector.memset(eps_t[:], float(eps))
            nc.scalar.activation(std[:], var[:], AF.Sqrt, bias=eps_t[:gps, :])
            # st2 = [rstd | -mean*rstd]
            st2 = cpool.tile([gps, 2 * cs], F32)
            nc.vector.reciprocal(st2[:, 0:cs], std[:])
            nc.vector.tensor_tensor(st2[:, cs:2 * cs], mean[:], st2[:, 0:cs],
                                    ALU.mult)
            nc.vector.tensor_scalar_mul(st2[:, cs:2 * cs], st2[:, cs:2 * cs],
                                        -1.0)
            # broadcast to channels: [P, 2cs] = gm2^T @ st2
            psb = ps1.tile([P, 2 * cs], F32)
            nc.tensor.matmul(psb[:], gm2[:], st2[:], start=True, stop=True)
            bc = cpool.tile([P, 2 * cs], F32)
            nc.vector.tensor_copy(bc[:], psb[:])
            # per-channel scale a = gw*rstd, bias b = gb + gw*(-mean*rstd)
            a_ch = cpool.tile([P, cs], F32)
            b_ch = cpool.tile([P, cs], F32)
            nc.vector.tensor_tensor(a_ch[:], gws[:], bc[:, 0:cs], ALU.mult)
            nc.vector.tensor_tensor(b_ch[:], gws[:], bc[:, cs:2 * cs], ALU.mult)
            nc.vector.tensor_tensor(b_ch[:], b_ch[:], gbs[:], ALU.add)

        if stop_after == 'p1':
            dbg = cpool.tile([P, 2 * cs], F32)
            nc.vector.tensor_copy(dbg[:, 0:cs], a_ch[:])
            nc.vector.tensor_copy(dbg[:, cs:2 * cs], b_ch[:])
            nc.sync.dma_start(out_d[0:P, 0:2 * cs], dbg[:])

        # ---------------- Phase 2: GN apply + q/k/v projections ----------------
        if stop_after != 'p1':
            for _rep in range(repeat):
                phase2(nc, tc, cs, c, n, pb, mm_dt, wq_d, wk_d, wv_d, x_d,
                       q_dram, k_full, vt_full, a_ch, b_ch, bqs, bks,
                       wp_t, wp_d, prefetch_wp=(_rep == 0))
                if stop_after is None and _rep < repeat - 1:
                    phase3(nc, tc, cs, c, n, qb, mm_dt, wp_t, x_d, out_d,
                           q_dram, k_full, vt_full, bps, ones_mat)

        if stop_after in ('p2', 'p2v'):
            with tc.tile_pool(name="dbg2", bufs=2) as dbg2:
                if stop_after == 'p2':
                    for ci in range(cs):
                        t = dbg2.tile([P, n], F32, tag="d")
                        nc.vector.tensor_copy(t[:], k_full[:, ci, :])
                        nc.sync.dma_start(out_d[ci * P:(ci + 1) * P, :], t[:])
                else:
                    for kt in range(n // P):
                        t = dbg2.tile([P, c], F32, tag="d")
                        nc.vector.tensor_copy(t[:], vt_full[:, kt, :])
                        nc.sync.dma_start(
                            out_d[:, kt * P:(kt + 1) * P].rearrange(
                                "c p -> p c"), t[:])

        # ---------------- Phase 3: attention + output projection ----------------
        if stop_after is None:
            phase3(nc, tc, cs, c, n, qb, mm_dt, wp_t, x_d, out_d, q_dram,
                   k_full, vt_full, bps, ones_mat)

        for p in (dpool, wppool, vpool, kpool, cpool):
            p.release()

    nc.finalize()
    return nc


def phase2(nc, tc, cs, c, n, pb, mm_dt, wq_d, wk_d, wv_d, x_d, q_dram,
           k_full, vt_full, a_ch, b_ch, bqs, bks, wp_t, wp_d,
           prefetch_wp=True):
    nbp = n // pb
    with tc.tile_pool(name="wqkv", bufs=1) as wpool, \
             tc.tile_pool(name="p2h", bufs=2) as p2h, \
             tc.tile_pool(name="p2x", bufs=4) as p2x, \
             tc.tile_pool(name="p2e", bufs=3) as p2e, \
             tc.tile_pool(name="ps2", bufs=4, space="PSUM") as ps2:
            wq = wpool.tile([P, cs, c], mm_dt)
            wk = wpool.tile([P, cs, c], mm_dt)
            wv = wpool.tile([P, cs, c], mm_dt)
            nc.sync.dma_start(wq[:], wq_d.rearrange("(ci p) o -> p ci o", p=P))
            nc.sync.dma_start(wk[:], wk_d.rearrange("(ci p) o -> p ci o", p=P))
            nc.sync.dma_start(wv[:], wv_d.rearrange("(ci p) o -> p ci o", p=P))

            for j in range(nbp):
                if j == min(1, nbp - 1) and prefetch_wp:
                    # prefetch the output-projection weight while the DMA
                    # queues are light, so phase 3 doesn't stall on it
                    nc.sync.dma_start(
                        wp_t[:], wp_d.rearrange("(ci p) o -> p ci o", p=P))
                hs = []
                for ci in range(cs):
                    xt = p2x.tile([P, pb], F32, tag="x2")
                    nc.sync.dma_start(
                        xt[:], x_d[ci * P:(ci + 1) * P, j * pb:(j + 1) * pb])
                    ht = p2h.tile([P, pb], mm_dt, tag=f"h{ci}")
                    nc.scalar.activation(ht[:], xt[:], AF.Identity,
                                         bias=b_ch[:, ci:ci + 1],
                                         scale=a_ch[:, ci:ci + 1])
                    hs.append(ht)
                # q (spilled to DRAM) and k (resident)
                for w_t, bias_t, is_q in ((wq, bqs, True), (wk, bks, False)):
                    for co in range(cs):
                        psq = ps2.tile([P, pb], F32, tag="proj")
                        for ci in range(cs):
                            nc.tensor.matmul(
                                psq[:], w_t[:, ci, co * P:(co + 1) * P],
                                hs[ci][:], start=(ci == 0), stop=(ci == cs - 1))
                        if is_q:
                            qe = p2e.tile([P, pb], mm_dt, tag="qe")
                            nc.scalar.activation(qe[:], psq[:], AF.Identity,
                                                 bias=bias_t[:, co:co + 1])
                            nc.sync.dma_start(
                                q_dram[:, co, j * pb:(j + 1) * pb], qe[:])
                        else:
                            nc.scalar.activation(
                                k_full[:, co, j * pb:(j + 1) * pb], psq[:],
                                AF.Identity, bias=bias_t[:, co:co + 1])
                # v^T tiles (bias folded into bps host-side)
                for pt in range(pb // P):
                    psv = ps2.tile([P, c], F32, tag="proj")
                    for ci in range(cs):
                        nc.tensor.matmul(
                            psv[:], hs[ci][:, pt * P:(pt + 1) * P], wv[:, ci, :],
                            start=(ci == 0), stop=(ci == cs - 1))
                    nc.vector.tensor_copy(
                        vt_full[:, j * (pb // P) + pt, :], psv[:])


def phase3(nc, tc, cs, c, n, qb, mm_dt, wp, x_d, out_d, q_dram,
           k_full, vt_full, bps, ones_mat):
    nqb = n // qb
    kt_n = n // P
    with tc.tile_pool(name="epool", bufs=1) as epool, \
             tc.tile_pool(name="p3", bufs=2) as p3, \
             tc.tile_pool(name="p3s", bufs=2) as p3s, \
             tc.tile_pool(name="pss", bufs=3, space="PSUM") as pss, \
             tc.tile_pool(name="pso", bufs=2, space="PSUM") as pso, \
             tc.tile_pool(name="psm", bufs=1, space="PSUM") as psm:
            for q_i in range(nqb):
                qs = slice(q_i * qb, (q_i + 1) * qb)
                qt = p3.tile([P, cs, qb], mm_dt, tag="qblk")
                nc.sync.dma_start(qt[:], q_dram[:, :, qs])
                # scores^T -> exp
                es = []
                for kt in range(kt_n):
                    ps_s = pss.tile([P, qb], F32, tag="s")
                    for co in range(cs):
                        nc.tensor.matmul(
                            ps_s[:], k_full[:, co, kt * P:(kt + 1) * P],
                            qt[:, co, :], start=(co == 0), stop=(co == cs - 1))
                    e_t = epool.tile([P, qb], mm_dt, tag=f"e{kt}")
                    nc.scalar.activation(e_t[:], ps_s[:], AF.Exp)
                    es.append(e_t)
                # softmax denominators: accumulate E on DVE (overlaps with
                # exp), then one fp32 ones-matmul replicates the partition
                # sums across all 128 rows; reciprocal reads PSUM directly.
                tsum = p3s.tile([P, qb], F32, tag="tsum")
                nc.vector.tensor_tensor(tsum[:], es[0][:], es[1][:], ALU.add)
                for kt in range(2, kt_n):
                    nc.vector.tensor_tensor(tsum[:], tsum[:], es[kt][:],
                                            ALU.add)
                rrep = p3s.tile([P, qb], F32, tag="rrep")
                # attn @ v -> [c, q_pos], normalized on evacuation
                att = p3.tile([P, cs, qb], mm_dt, tag="att")
                for co in range(cs):
                    ps_o = pso.tile([P, qb], F32, tag="o")
                    for kt in range(kt_n):
                        nc.tensor.matmul(
                            ps_o[:], vt_full[:, kt, co * P:(co + 1) * P],
                            es[kt][:], start=(kt == 0), stop=(kt == kt_n - 1))
                    if co == 0:
                        ps_sum = psm.tile([P, qb], F32, tag="sum")
                        nc.tensor.matmul(ps_sum[:], ones_mat[:], tsum[:],
                                         start=True, stop=True)
                        nc.vector.reciprocal(rrep[:], ps_sum[:])
                    nc.vector.tensor_tensor(att[:, co, :], ps_o[:], rrep[:],
                                            ALU.mult)
                # output projection + bias + residual
                for co in range(cs):
                    ps_p = pso.tile([P, qb], F32, tag="p")
                    for ci in range(cs):
                        nc.tensor.matmul(
                            ps_p[:], wp[:, ci, co * P:(co + 1) * P],
                            att[:, ci, :], start=(ci == 0), stop=(ci == cs - 1))
                    xr = p3s.tile([P, qb], F32, tag="xr")
                    nc.sync.dma_start(xr[:], x_d[co * P:(co + 1) * P, qs])
                    t1 = p3s.tile([P, qb], F32, tag="t1")
                    nc.vector.tensor_tensor(t1[:], ps_p[:], xr[:], ALU.add)
                    ot = p3s.tile([P, qb], F32, tag="ot")
                    nc.scalar.activation(ot[:], t1[:], AF.Identity,
                                         bias=bps[:, co:co + 1])
                    nc.sync.dma_start(out_d[co * P:(co + 1) * P, qs], ot[:])


def _prep_host_inputs(x, gn_weight, gn_bias, wq, bq, wk, bk, wv, bv, wp, bp,
                      c=512):
    """Host-side weight prep shared by all cores."""
    cs = c // P
    scale = 1.0 / np.sqrt(c)

    def stripe(v):  # [c] -> [P, cs] with v[ci*128 + p] at [p, ci]
        return np.ascontiguousarray(
            v.reshape(cs, P).T.astype(np.float32))

    if USE_V2:
        f8 = ml_dtypes.float8_e4m3
        common = {
            "wqt": np.ascontiguousarray((np.asarray(wq).T * SW).astype(f8)),
            "wkt": np.ascontiguousarray((np.asarray(wk).T * SW).astype(f8)),
            "wvt": np.ascontiguousarray((np.asarray(wv).T * SW).astype(f8)),
            "wpt": np.ascontiguousarray((np.asarray(wp).T * SW).astype(f8)),
            "bqs": stripe(bq),
            "bks": stripe(bk),
            "bps": stripe(bp + wp.astype(np.float64) @ bv.astype(np.float64)),
            "gws": stripe(gn_weight),
            "gbs": stripe(gn_bias),
        }
    else:
        common = {
            "wqt": np.ascontiguousarray((np.asarray(wq).T * scale).astype(MM_NP)),
            "wkt": np.ascontiguousarray(np.asarray(wk).T.astype(MM_NP)),
            "wvt": np.ascontiguousarray(np.asarray(wv).T.astype(MM_NP)),
            "wpt": np.ascontiguousarray(np.asarray(wp).T.astype(MM_NP)),
            "bqs": stripe(bq * scale),
            "bks": stripe(bk),
            "bps": stripe(bp + wp.astype(np.float64) @ bv.astype(np.float64)),
            "gws": stripe(gn_weight),
            "gbs": stripe(gn_bias),
        }
    gsize = 16  # channels per group (512/32)
    gps = P // gsize
    gmat = np.zeros((P, gps), np.float32)
    gmat[np.arange(P), np.arange(P) // gsize] = 1.0
    common["gmat"] = gmat
    common["gmat2"] = np.ascontiguousarray(gmat.T)
    return common


_NC_CACHE = {}


def kernel(x, gn_weight, gn_bias, wq, bq, wk, bk, wv, bv, wp, bp):
    b, c, h, w = x.shape
    n = h * w
    key = (c, n)
    if key not in _NC_CACHE:
        _NC_CACHE[key] = build_attn_kernel(c=c, n=n)
    nc = _NC_CACHE[key]

    common = _prep_host_inputs(x, gn_weight, gn_bias, wq, bq, wk, bk, wv, bv,
                               wp, bp, c=c)
    xf = np.ascontiguousarray(np.asarray(x, np.float32).reshape(b, c, n))
    in_maps = [{**common, "x": xf[i]} for i in range(b)]
    res = bass_utils.run_bass_kernel_spmd(nc, in_maps, core_ids=list(range(b)))
    out = np.stack([r["out"] for r in res.results])
    return out.reshape(b, c, h, w).astype(np.float32)

